# revision 33
# baseline (speedup 1.0000x reference)
"""Trainium2 Bass kernel for bilinear causal attention (no softmax).

Math (from the reference):
  Omega[b,h,t,u] = r_t^T Q^h r_u            (scores)
  out[b,t,:]     = sum_h sum_{u<=t} Omega[b,h,t,u] * (E^h r_u)

Because there is no softmax the contraction is linear in Omega, so we use
the chunked linear-attention identity instead of materializing the full
[2048, 2048] score matrix.  With K = r Q (per head), V = r E^T:

  out[t] = sum_h [ K_h[t] @ M_h(c)  +  sum_{u in chunk(t), u<=t} S[t,u] V_h[u] ]
  M_h(c) = sum_{u < chunk_start(c)} r[u] (x) V_h[u]     ([256, 256] state)

Shapes: r_prime [1,4,2048,256] f32, Q [1,8,256,256], E [1,8,256,256],
out [1,4,2048,256] f32.

Sharding over 8 NeuronCores: core = 2*b + hg handles batch b (4 batches)
and head-group hg (heads hg*4 .. hg*4+3).  Each core produces a partial
output summed over its 4 heads; the host adds the two head-group
partials per batch.  No on-chip collectives.

Per-core algorithm (bf16 matmuls, f32 PSUM):
  Phase A (identical to the quadratic kernel):
    KT[h]: KT[j,t] = sum_i Q[i,j] rT[i,t]      ([256,2048] per head)
    V[hp]: V[u,i'] = sum_j rT[j,u] ET[j,i']    ([2048,512] per head pair)
  Phase B, per 256-wide t-chunk c (8 chunks):
    ST   : ST[u,t] = sum_j rT[j,u] KT[j,t] for the two diagonal u-tiles;
           the two diagonal 128x128 blocks are causally masked on DVE,
           the full off-diagonal block is copied on ACT, all into SBUF bf16
    inter: OT[i',t] += M[j,i']^T KT[j,t]   (state contribution, c>0)
    G    : state[j,i'] += sum_{u in c} r[u,j] V[u,i']  (persistent PSUM
           accumulation across chunks; copied to SBUF bf16 as M each chunk)
    intra: OT[i',t] += V[u,i']^T ST_masked[u,t]
  Output is produced transposed ([i',t]) and transposed back on the host.

GPSIMD cannot access PSUM, so all PSUM reads ride DVE (vector) or ACT
(scalar); gpsimd only issues DMAs.
"""

import numpy as np
import ml_dtypes

N_T = 2048           # sequence length t
N_IN = 256           # feature dim (i, j, i' all 256)
CB = 256             # phase-B chunk width (t columns per chunk)
NCH = N_T // CB      # 8 chunks
ACH = 512            # phase-A moving-operand chunk (one f32 PSUM bank)
TQ = N_T // 128      # 16 row tiles of 128
N_CORES = 8

_CACHE = {}


def _build_nc():
    from concourse import mybir, bacc, tile

    BF16 = mybir.dt.bfloat16
    F32 = mybir.dt.float32

    nc = bacc.Bacc(
        "TRN2", target_bir_lowering=False, debug=False, num_devices=N_CORES
    )
    # All inputs partition-major so each loads with few large DMAs.
    rt_d = nc.dram_tensor("rt", [128, 2, N_T], BF16, kind="ExternalInput").ap()
    rn_d = nc.dram_tensor("rn", [128, TQ, N_IN], BF16, kind="ExternalInput").ap()
    q_d = nc.dram_tensor("q", [128, 4, 2, N_IN], BF16, kind="ExternalInput").ap()
    # et pairs two heads side by side: [p, head-pair, j-chunk, 512]
    et_d = nc.dram_tensor("et", [128, 2, 2, ACH], BF16, kind="ExternalInput").ap()
    # triu(ones(128,128)): valid (u<=t) for a diagonal [u,t] block
    mask_d = nc.dram_tensor("cmask", [128, 128], F32, kind="ExternalInput").ap()
    # transposed output: [i' chunk, i' in chunk, t]
    out_d = nc.dram_tensor("out", [2, 128, N_T], F32, kind="ExternalOutput").ap()

    with tile.TileContext(nc) as tc:
        with (
            tc.tile_pool(name="consts", bufs=1) as consts,
            tc.tile_pool(name="sbw", bufs=1) as sbw,
            tc.tile_pool(name="outsb", bufs=3) as outp,
            tc.tile_pool(name="psum", bufs=1, space="PSUM") as psum,
        ):
            rt_sb = consts.tile([128, 2, N_T], BF16)
            rn_sb = consts.tile([128, TQ, N_IN], BF16)
            q_sb = consts.tile([128, 4, 2, N_IN], BF16)
            et_sb = consts.tile([128, 2, 2, ACH], BF16)
            mask_sb = consts.tile([128, 128], F32)
            kt_sb = consts.tile([128, 4, 2, N_T], BF16)
            # [p, head-pair, u-tile, (head-in-pair x i')]
            v_sb = consts.tile([128, 2, TQ, 2 * N_IN], BF16)
            # M state snapshot, bf16: [p(j in tile), j-tile, head-pair,
            # (head-in-pair x i')]
            m_sb = consts.tile([128, 2, 2, 512], BF16)

            # Input DMAs. gpsimd's SWDGE queue measures ~170 GB/s vs the
            # HWDGE queues' ~52 GB/s, so the first-needed tensors (q head
            # 0, rt chunks, et) go through gpsimd in need-order; the rest
            # spills to sync/scalar.  rn (natural-layout r, G stationary)
            # is only needed once phase B starts, so it rides the queue
            # tails.
            def _rt_dma(eng, tcn):
                eng.dma_start(
                    out=rt_sb[:, :, tcn * ACH : (tcn + 1) * ACH],
                    in_=rt_d[:, :, tcn * ACH : (tcn + 1) * ACH],
                )

            # Need-order per queue (observed queue-start lags: sync ~8.1us,
            # scalar ~9.2us, gpsimd ~9.9us; rates ~88/111/122 B/ns).
            # rt chunk 0 split in half so the first KT matmul can start
            # after only 128KB has landed.
            nc.sync.dma_start(
                out=rt_sb[:, 0, 0:ACH], in_=rt_d[:, 0, 0:ACH]
            )
            nc.sync.dma_start(
                out=rt_sb[:, 1, 0:ACH], in_=rt_d[:, 1, 0:ACH]
            )
            _rt_dma(nc.sync, 1)
            nc.sync.dma_start(out=q_sb[:, 3], in_=q_d[:, 3])
            nc.sync.dma_start(out=rn_sb[:, 0:8], in_=rn_d[:, 0:8])
            nc.scalar.dma_start(out=q_sb[:, 0], in_=q_d[:, 0])
            _rt_dma(nc.scalar, 2)
            nc.scalar.dma_start(out=q_sb[:, 2], in_=q_d[:, 2])
            nc.scalar.dma_start(out=rn_sb[:, 8:16], in_=rn_d[:, 8:16])
            _rt_dma(nc.gpsimd, 3)
            nc.gpsimd.dma_start(out=q_sb[:, 1], in_=q_d[:, 1])
            nc.gpsimd.dma_start(out=et_sb[:, 0], in_=et_d[:, 0])
            nc.gpsimd.dma_start(out=et_sb[:, 1], in_=et_d[:, 1])
            nc.gpsimd.dma_start(out=mask_sb[:], in_=mask_d[:])

            # Persistent PSUM accumulators for the linear-attention state:
            # state[hp][jt][p(j), (sh,i')] accumulates G across chunks.
            state = [
                [
                    psum.tile(
                        [128, 512], F32, tag="state", bufs=4,
                        name=f"state_{hp}_{jt}",
                    )
                    for jt in range(2)
                ]
                for hp in range(2)
            ]

            # Phase-A psum tiles alternate between the "work" and "ot"
            # tags so four banks rotate during phase A even though each
            # tag only owns two.
            ai = [0]

            def _apsum(name):
                tag = "work" if ai[0] % 2 == 0 else "ot"
                ai[0] += 1
                return psum.tile([128, ACH], F32, tag=tag, bufs=2, name=name)

            # PSUM->SBUF copies alternate DVE / ACT (gpsimd cannot touch
            # PSUM).
            cp_i = [0]

            def _cp(out, in_):
                if cp_i[0] % 2 == 0:
                    nc.vector.tensor_copy(out, in_)
                else:
                    nc.scalar.copy(out, in_)
                cp_i[0] += 1

            # PE warm-up: junk matmuls on zeroed SBUF fill the DMA wait and
            # lift the HAM clock gate before real work arrives.  All of
            # them accumulate into a single PSUM tile so the warm-up never
            # stalls on PSUM slot reuse.
            junk_sb = consts.tile([128, 640], BF16)
            nc.vector.memset(junk_sb[:], 0.0)
            junk_ps = _apsum("junk_ps")
            NJUNK = 7
            for jn in range(NJUNK):
                nc.tensor.matmul(
                    junk_ps[:], junk_sb[:, 0:128], junk_sb[:, 128:640],
                    start=(jn == 0), stop=(jn == NJUNK - 1),
                )

            # Phase A: per head, KT = (rQ)^T; per head-pair, V = r @ E^T
            # (two heads share the 512-wide moving operand). Kept in SBUF
            # as bf16. Emission order tracks DMA arrival (PE executes its
            # instruction stream in order): KT h0, KT h1, V pair0, ...
            def _kt(h, tc_order=(0, 1, 2, 3), split_tc0=False, pair_order=None):
                pairs = pair_order or [
                    (ipc, tcn) for ipc in range(2) for tcn in tc_order
                ]
                for ipc, tcn in pairs:
                        kt_ps = _apsum("kt_ps")
                        if split_tc0 and tcn == 0:
                            # 256-wide sub-groups so the first matmul only
                            # needs the first 64KB rt piece.
                            for half in range(2):
                                for ic in range(2):
                                    nc.tensor.matmul(
                                        kt_ps[:, half * 256 : (half + 1) * 256],
                                        q_sb[:, h, ic, ipc * 128 : (ipc + 1) * 128],
                                        rt_sb[:, ic, half * 256 : (half + 1) * 256],
                                        start=(half == 0 and ic == 0),
                                        stop=(half == 1 and ic == 1),
                                    )
                        else:
                            for ic in range(2):
                                nc.tensor.matmul(
                                    kt_ps[:],
                                    q_sb[:, h, ic, ipc * 128 : (ipc + 1) * 128],
                                    rt_sb[:, ic, tcn * ACH : (tcn + 1) * ACH],
                                    start=(ic == 0),
                                    stop=(ic == 1),
                                )
                        _cp(
                            kt_sb[:, h, ipc, tcn * ACH : (tcn + 1) * ACH],
                            kt_ps[:],
                        )

            def _v(hp):
                for ut in range(TQ):
                    v_ps = _apsum("v_ps")
                    for ic in range(2):
                        nc.tensor.matmul(
                            v_ps[:],
                            rt_sb[:, ic, ut * 128 : (ut + 1) * 128],
                            et_sb[:, hp, ic, :],
                            start=(ic == 0),
                            stop=(ic == 1),
                        )
                    _cp(v_sb[:, hp, ut, :], v_ps[:])

            # rt1 is the last rt chunk to land, so head 0 consumes it in
            # its final two psum groups.
            _kt(0, pair_order=[(0, 0), (0, 3), (0, 2), (1, 0), (1, 3), (1, 2),
                               (0, 1), (1, 1)])
            _kt(1, tc_order=(0, 3, 2, 1))
            _v(0)
            _kt(2)
            _kt(3)
            _v(1)

            # ---- Phase B: chunked linear attention -------------------
            # ST psum layout per head: cols 0:128 = u0 x t[0:128] (diag),
            # 128:256 = u0 x t[128:256] (full), 256:384 = u1 x t[128:256]
            # (diag).
            def _st(c, h):
                st_ps = psum.tile(
                    [128, 384], F32, tag="work", bufs=2, name="st_ps"
                )
                u0, u1 = 2 * c, 2 * c + 1
                for jt in range(2):
                    nc.tensor.matmul(
                        st_ps[:, 0:256],
                        rt_sb[:, jt, u0 * 128 : (u0 + 1) * 128],
                        kt_sb[:, h, jt, c * CB : c * CB + 256],
                        start=(jt == 0),
                        stop=False,
                    )
                for jt in range(2):
                    nc.tensor.matmul(
                        st_ps[:, 256:384],
                        rt_sb[:, jt, u1 * 128 : (u1 + 1) * 128],
                        kt_sb[:, h, jt, c * CB + 128 : c * CB + 256],
                        start=False,
                        stop=(jt == 1),
                    )
                st_sb = sbw.tile([128, 384], BF16, tag="st", bufs=8, name="st_sb")
                nc.vector.tensor_mul(
                    st_sb[:, 0:128], st_ps[:, 0:128], mask_sb[:]
                )
                nc.scalar.copy(st_sb[:, 128:256], st_ps[:, 128:256])
                nc.vector.tensor_mul(
                    st_sb[:, 256:384], st_ps[:, 256:384], mask_sb[:]
                )
                return st_sb

            def _inter(c, h, ot, first):
                hp, sh = h // 2, h % 2
                for jt in range(2):
                    for it in range(2):
                        nc.tensor.matmul(
                            ot[:, it * 256 : (it + 1) * 256],
                            m_sb[:, jt, hp, sh * 256 + it * 128 : sh * 256 + (it + 1) * 128],
                            kt_sb[:, h, jt, c * CB : (c + 1) * CB],
                            start=(first and jt == 0 and it == 0),
                            stop=False,
                        )

            def _ointra(c, h, ot, st_sb, first, last):
                hp, sh = h // 2, h % 2
                u0, u1 = 2 * c, 2 * c + 1
                for it in range(2):
                    nc.tensor.matmul(
                        ot[:, it * 256 : it * 256 + 256],
                        v_sb[:, hp, u0, sh * 256 + it * 128 : sh * 256 + (it + 1) * 128],
                        st_sb[:, 0:256],
                        start=(first and it == 0),
                        stop=False,
                    )
                    nc.tensor.matmul(
                        ot[:, it * 256 + 128 : it * 256 + 256],
                        v_sb[:, hp, u1, sh * 256 + it * 128 : sh * 256 + (it + 1) * 128],
                        st_sb[:, 256:384],
                        start=False,
                        stop=(last and it == 1),
                    )

            def _inter_last(c, h, otA, otB, first):
                # Last chunk: OT halves live in two separate PSUM banks so
                # the first half can drain to DRAM while the second half
                # is still accumulating (no PSUM bank-level WAR).
                hp, sh = h // 2, h % 2
                for jt in range(2):
                    for it in range(2):
                        tgt = otA if it == 0 else otB
                        nc.tensor.matmul(
                            tgt[:, 0:256],
                            m_sb[:, jt, hp, sh * 256 + it * 128 : sh * 256 + (it + 1) * 128],
                            kt_sb[:, h, jt, c * CB : (c + 1) * CB],
                            start=(first and jt == 0),
                            stop=False,
                        )

            for c in range(NCH):
                last = c == NCH - 1
                if not last:
                    ot = psum.tile([128, 512], F32, tag="ot", bufs=2, name="ot")
                else:
                    otA = psum.tile([128, 256], F32, tag="ot", bufs=2, name="otA")
                    otB = psum.tile([128, 256], F32, tag="ot", bufs=2, name="otB")
                st_tiles = [None] * 4
                # ST for h0/h1 first; h2/h3 interleave with inter so the
                # DVE/ACT mask ops have a window before intra needs them.
                st_tiles[0] = _st(c, 0)
                st_tiles[1] = _st(c, 1)
                if c > 0 and not last:
                    _inter(c, 0, ot, first=True)
                    st_tiles[2] = _st(c, 2)
                    _inter(c, 1, ot, first=False)
                    st_tiles[3] = _st(c, 3)
                    _inter(c, 2, ot, first=False)
                    _inter(c, 3, ot, first=False)
                elif last:
                    _inter_last(c, 0, otA, otB, first=True)
                    st_tiles[2] = _st(c, 2)
                    _inter_last(c, 1, otA, otB, first=False)
                    st_tiles[3] = _st(c, 3)
                    _inter_last(c, 2, otA, otB, first=False)
                    _inter_last(c, 3, otA, otB, first=False)
                else:
                    st_tiles[2] = _st(c, 2)
                    st_tiles[3] = _st(c, 3)

                if c < NCH - 1:
                    # G: accumulate this chunk into the persistent state,
                    # then snapshot state -> M (bf16 SBUF) for the next
                    # chunk's inter matmuls.
                    for hp in range(2):
                        for jt in range(2):
                            for us in range(2):
                                ut = 2 * c + us
                                nc.tensor.matmul(
                                    state[hp][jt][:],
                                    rn_sb[:, ut, jt * 128 : (jt + 1) * 128],
                                    v_sb[:, hp, ut, :],
                                    start=(c == 0 and us == 0),
                                    stop=(c == NCH - 2 and us == 1),
                                )
                    # hp0 copies on ACT (inter h0/h1 read them first),
                    # hp1 on DVE.
                    nc.scalar.copy(m_sb[:, 0, 0, :], state[0][0][:])
                    nc.scalar.copy(m_sb[:, 1, 0, :], state[0][1][:])
                    nc.vector.tensor_copy(m_sb[:, 0, 1, :], state[1][0][:])
                    nc.vector.tensor_copy(m_sb[:, 1, 1, :], state[1][1][:])

                if c < NCH - 1:
                    for h in range(4):
                        _ointra(
                            c, h, ot, st_tiles[h],
                            first=(c == 0 and h == 0),
                            last=(h == 3),
                        )
                    o_sb = outp.tile([128, 512], F32, tag="osb", name="o_sb")
                    nc.scalar.copy(o_sb[:], ot[:])
                    nc.sync.dma_start(
                        out=out_d[0, :, c * CB : (c + 1) * CB],
                        in_=o_sb[:, 0:256],
                    )
                    nc.gpsimd.dma_start(
                        out=out_d[1, :, c * CB : (c + 1) * CB],
                        in_=o_sb[:, 256:512],
                    )
                else:
                    # Last chunk: the end-of-kernel barrier waits on the
                    # final output DMA, so drain the first OT half (its
                    # own PSUM bank) as soon as its intra matmuls retire
                    # (it-major order) and spread the copies/DMAs over
                    # both PSUM-capable engines and four DMA queues.
                    o_sb = outp.tile([128, 512], F32, tag="osb", name="o_sb")
                    u0, u1 = 2 * c, 2 * c + 1
                    for it in range(2):
                        tgt = otA if it == 0 else otB
                        for h in range(4):
                            hp, sh = h // 2, h % 2
                            i0 = sh * 256 + it * 128
                            nc.tensor.matmul(
                                tgt[:, 0:256],
                                v_sb[:, hp, u0, i0 : i0 + 128],
                                st_tiles[h][:, 0:256],
                                start=False,
                                stop=False,
                            )
                            nc.tensor.matmul(
                                tgt[:, 128:256],
                                v_sb[:, hp, u1, i0 : i0 + 128],
                                st_tiles[h][:, 256:384],
                                start=False,
                                stop=(h == 3),
                            )
                    # The end-of-kernel drain waits on every DMA queue that
                    # was used; gpsimd's SWDGE drains slowest (~2.5us), so
                    # the final DMAs ride only the sync/scalar HWDGE queues.
                        if it == 0:
                            nc.vector.tensor_copy(
                                o_sb[:, 0:256], otA[:, 0:256]
                            )
                            nc.sync.dma_start(
                                out=out_d[0, :, c * CB : c * CB + 128],
                                in_=o_sb[:, 0:128],
                            )
                            nc.scalar.dma_start(
                                out=out_d[0, :, c * CB + 128 : (c + 1) * CB],
                                in_=o_sb[:, 128:256],
                            )
                    nc.vector.tensor_copy(o_sb[:, 256:512], otB[:, 0:256])
                    nc.scalar.dma_start(
                        out=out_d[1, :, c * CB : c * CB + 128],
                        in_=o_sb[:, 256:384],
                    )
                    nc.sync.dma_start(
                        out=out_d[1, :, c * CB + 128 : (c + 1) * CB],
                        in_=o_sb[:, 384:512],
                    )

    nc.compile()
    return nc


def _get_nc():
    if "nc" not in _CACHE:
        _CACHE["nc"] = _build_nc()
    return _CACHE["nc"]


def _make_in_maps(r_prime, Q, E):
    bf16 = ml_dtypes.bfloat16
    cmask = np.triu(np.ones((128, 128), np.float32))
    in_maps = []
    for core in range(N_CORES):
        b, hg = core // 2, core % 2
        r = r_prime[0, b]  # [2048, 256]
        # rt[p, ic, t] = r[t, ic*128+p]
        rt = np.ascontiguousarray(
            r.T.reshape(2, 128, N_T).transpose(1, 0, 2)
        ).astype(bf16)
        # rn[p, ut, j] = r[ut*128+p, j]
        rn = np.ascontiguousarray(
            r.reshape(TQ, 128, N_IN).transpose(1, 0, 2)
        ).astype(bf16)
        # q[p, h, ic, f] = Q[h, ic*128+p, f]
        qh = np.ascontiguousarray(
            Q[0, hg * 4 : hg * 4 + 4]
            .reshape(4, 2, 128, N_IN)
            .transpose(2, 0, 1, 3)
        ).astype(bf16)
        # et[p, hp, jc, sh*256+f] = E[2hp+sh].T[jc*128+p, f]
        eth = (
            E[0, hg * 4 : hg * 4 + 4]
            .transpose(0, 2, 1)  # [h, j, i']
            .reshape(2, 2, 2, 128, N_IN)  # [hp, sh, jc, p, f]
            .transpose(3, 0, 2, 1, 4)  # [p, hp, jc, sh, f]
            .reshape(128, 2, 2, ACH)
        )
        eth = np.ascontiguousarray(eth).astype(bf16)
        in_maps.append(
            {"rt": rt, "rn": rn, "q": qh, "et": eth, "cmask": cmask}
        )
    return in_maps


def _ensure_ntff_hook():
    """The container's `antenv` stub lacks `axon_hooks`, so the boot-time
    NTFF profile hook registration silently no-ops. Recreate it so
    trace=True yields exec_time_ns. Only used by the test harness."""
    import sys
    import types

    if "antenv.axon_hooks" not in sys.modules:
        import antenv

        mod = types.ModuleType("antenv.axon_hooks")
        state = {}
        mod.set_axon_ntff_profile_hook = lambda h: state.update(h=h)
        mod.get_axon_ntff_profile_hook = lambda: state.get("h")
        sys.modules["antenv.axon_hooks"] = mod
        antenv.axon_hooks = mod
    from antenv.axon_hooks import (
        get_axon_ntff_profile_hook,
        set_axon_ntff_profile_hook,
    )

    if get_axon_ntff_profile_hook() is None:
        from trn_agent_boot.trn_boot import _ntff_profile_via_ctypes

        set_axon_ntff_profile_hook(
            _ntff_profile_via_ctypes("/opt/axon/libaxon_pjrt.so")
        )


def _run(r_prime, Q, E, trace=False, trace_kwargs=None):
    from concourse.bass_utils import run_bass_kernel_spmd

    try:
        _ensure_ntff_hook()
    except Exception:
        pass  # profiling is optional; never block the actual run
    r_prime = np.asarray(r_prime, dtype=np.float32)
    Q = np.asarray(Q, dtype=np.float32)
    E = np.asarray(E, dtype=np.float32)
    in_maps = _make_in_maps(r_prime, Q, E)
    nc = _get_nc()
    kw = {}
    if trace:
        kw["trace"] = True
        if trace_kwargs:
            kw.update(trace_kwargs)
    res = run_bass_kernel_spmd(nc, in_maps, core_ids=list(range(N_CORES)), **kw)
    out = np.zeros((1, 4, N_T, N_IN), np.float32)
    for b in range(4):
        p0 = np.asarray(res.results[2 * b]["out"], np.float32).reshape(N_IN, N_T)
        p1 = np.asarray(res.results[2 * b + 1]["out"], np.float32).reshape(
            N_IN, N_T
        )
        out[0, b] = (p0 + p1).T
    return out, res


def kernel(r_prime, Q, E):
    out, _ = _run(r_prime, Q, E, trace=False)
    return out


# revision 34
# speedup vs baseline: 1.0317x; 1.0317x over previous
"""Trainium2 Bass kernel for bilinear causal attention (no softmax).

Math (from the reference):
  Omega[b,h,t,u] = r_t^T Q^h r_u            (scores)
  out[b,t,:]     = sum_h sum_{u<=t} Omega[b,h,t,u] * (E^h r_u)

Because there is no softmax the contraction is linear in Omega, so we use
the chunked linear-attention identity instead of materializing the full
[2048, 2048] score matrix.  With K = r Q (per head), V = r E^T:

  out[t] = sum_h [ K_h[t] @ M_h(c)  +  sum_{u in chunk(t), u<=t} S[t,u] V_h[u] ]
  M_h(c) = sum_{u < chunk_start(c)} r[u] (x) V_h[u]     ([256, 256] state)

Shapes: r_prime [1,4,2048,256] f32, Q [1,8,256,256], E [1,8,256,256],
out [1,4,2048,256] f32.

Sharding over 8 NeuronCores: core = 2*b + hg handles batch b (4 batches)
and head-group hg (heads hg*4 .. hg*4+3).  Each core produces a partial
output summed over its 4 heads; the host adds the two head-group
partials per batch.  No on-chip collectives.

Per-core algorithm (bf16 matmuls, f32 PSUM):
  Phase A (identical to the quadratic kernel):
    KT[h]: KT[j,t] = sum_i Q[i,j] rT[i,t]      ([256,2048] per head)
    V[hp]: V[u,i'] = sum_j rT[j,u] ET[j,i']    ([2048,512] per head pair)
  Phase B, per 256-wide t-chunk c (8 chunks):
    ST   : ST[u,t] = sum_j rT[j,u] KT[j,t] for the two diagonal u-tiles;
           the two diagonal 128x128 blocks are causally masked on DVE,
           the full off-diagonal block is copied on ACT, all into SBUF bf16
    inter: OT[i',t] += M[j,i']^T KT[j,t]   (state contribution, c>0)
    G    : state[j,i'] += sum_{u in c} r[u,j] V[u,i']  (persistent PSUM
           accumulation across chunks; copied to SBUF bf16 as M each chunk)
    intra: OT[i',t] += V[u,i']^T ST_masked[u,t]
  Output is produced transposed ([i',t]) and transposed back on the host.

GPSIMD cannot access PSUM, so all PSUM reads ride DVE (vector) or ACT
(scalar); gpsimd only issues DMAs.
"""

import numpy as np
import ml_dtypes

N_T = 2048           # sequence length t
N_IN = 256           # feature dim (i, j, i' all 256)
CB = 256             # phase-B chunk width (t columns per chunk)
NCH = N_T // CB      # 8 chunks
ACH = 512            # phase-A moving-operand chunk (one f32 PSUM bank)
TQ = N_T // 128      # 16 row tiles of 128
N_CORES = 8

_CACHE = {}


def _build_nc():
    from concourse import mybir, bacc, tile

    BF16 = mybir.dt.bfloat16
    F32 = mybir.dt.float32

    nc = bacc.Bacc(
        "TRN2", target_bir_lowering=False, debug=False, num_devices=N_CORES
    )
    # All inputs partition-major so each loads with few large DMAs.
    rt_d = nc.dram_tensor("rt", [128, 2, N_T], BF16, kind="ExternalInput").ap()
    rn_d = nc.dram_tensor("rn", [128, TQ, N_IN], BF16, kind="ExternalInput").ap()
    q_d = nc.dram_tensor("q", [128, 4, 2, N_IN], BF16, kind="ExternalInput").ap()
    # et pairs two heads side by side: [p, head-pair, j-chunk, 512]
    et_d = nc.dram_tensor("et", [128, 2, 2, ACH], BF16, kind="ExternalInput").ap()
    # triu(ones(128,128)): valid (u<=t) for a diagonal [u,t] block
    mask_d = nc.dram_tensor("cmask", [128, 128], F32, kind="ExternalInput").ap()
    # transposed output: [i' chunk, i' in chunk, t]
    out_d = nc.dram_tensor("out", [2, 128, N_T], F32, kind="ExternalOutput").ap()

    with tile.TileContext(nc) as tc:
        with (
            tc.tile_pool(name="consts", bufs=1) as consts,
            tc.tile_pool(name="sbw", bufs=1) as sbw,
            tc.tile_pool(name="outsb", bufs=3) as outp,
            tc.tile_pool(name="psum", bufs=1, space="PSUM") as psum,
        ):
            rt_sb = consts.tile([128, 2, N_T], BF16)
            rn_sb = consts.tile([128, TQ, N_IN], BF16)
            q_sb = consts.tile([128, 4, 2, N_IN], BF16)
            et_sb = consts.tile([128, 2, 2, ACH], BF16)
            mask_sb = consts.tile([128, 128], F32)
            kt_sb = consts.tile([128, 4, 2, N_T], BF16)
            # [p, head-pair, u-tile, (head-in-pair x i')]
            v_sb = consts.tile([128, 2, TQ, 2 * N_IN], BF16)
            # M state snapshot, bf16: [p(j in tile), j-tile, head-pair,
            # (head-in-pair x i')]
            m_sb = consts.tile([128, 2, 2, 512], BF16)

            # Input DMAs. gpsimd's SWDGE queue measures ~170 GB/s vs the
            # HWDGE queues' ~52 GB/s, so the first-needed tensors (q head
            # 0, rt chunks, et) go through gpsimd in need-order; the rest
            # spills to sync/scalar.  rn (natural-layout r, G stationary)
            # is only needed once phase B starts, so it rides the queue
            # tails.
            def _rt_dma(eng, tcn):
                eng.dma_start(
                    out=rt_sb[:, :, tcn * ACH : (tcn + 1) * ACH],
                    in_=rt_d[:, :, tcn * ACH : (tcn + 1) * ACH],
                )

            # Need-order per queue (observed queue-start lags: sync ~8.1us,
            # scalar ~9.2us, gpsimd ~9.9us; rates ~88/111/122 B/ns).
            # rt chunk 0 split in half so the first KT matmul can start
            # after only 128KB has landed.
            nc.sync.dma_start(
                out=rt_sb[:, 0, 0:ACH], in_=rt_d[:, 0, 0:ACH]
            )
            nc.sync.dma_start(
                out=rt_sb[:, 1, 0:ACH], in_=rt_d[:, 1, 0:ACH]
            )
            _rt_dma(nc.sync, 1)
            nc.sync.dma_start(out=q_sb[:, 3], in_=q_d[:, 3])
            nc.sync.dma_start(out=rn_sb[:, 0:8], in_=rn_d[:, 0:8])
            nc.scalar.dma_start(out=q_sb[:, 0], in_=q_d[:, 0])
            _rt_dma(nc.scalar, 2)
            nc.scalar.dma_start(out=q_sb[:, 2], in_=q_d[:, 2])
            nc.scalar.dma_start(out=rn_sb[:, 8:16], in_=rn_d[:, 8:16])
            _rt_dma(nc.gpsimd, 3)
            nc.gpsimd.dma_start(out=q_sb[:, 1], in_=q_d[:, 1])
            nc.gpsimd.dma_start(out=et_sb[:, 0], in_=et_d[:, 0])
            nc.gpsimd.dma_start(out=et_sb[:, 1], in_=et_d[:, 1])
            nc.gpsimd.dma_start(out=mask_sb[:], in_=mask_d[:])

            # Persistent PSUM accumulators for the linear-attention state:
            # state[hp][jt][p(j), (sh,i')] accumulates G across chunks.
            state = [
                [
                    psum.tile(
                        [128, 512], F32, tag="state", bufs=4,
                        name=f"state_{hp}_{jt}",
                    )
                    for jt in range(2)
                ]
                for hp in range(2)
            ]

            # Phase-A psum tiles alternate between the "work" and "ot"
            # tags so four banks rotate during phase A even though each
            # tag only owns two.
            ai = [0]

            def _apsum(name):
                tag = "work" if ai[0] % 2 == 0 else "ot"
                ai[0] += 1
                return psum.tile([128, ACH], F32, tag=tag, bufs=2, name=name)

            # PSUM->SBUF copies alternate DVE / ACT (gpsimd cannot touch
            # PSUM).
            cp_i = [0]

            def _cp(out, in_):
                if cp_i[0] % 2 == 0:
                    nc.vector.tensor_copy(out, in_)
                else:
                    nc.scalar.copy(out, in_)
                cp_i[0] += 1

            # PE warm-up: junk matmuls on zeroed SBUF fill the DMA wait and
            # lift the HAM clock gate before real work arrives.  All of
            # them accumulate into a single PSUM tile so the warm-up never
            # stalls on PSUM slot reuse.
            junk_sb = consts.tile([128, 640], BF16)
            nc.vector.memset(junk_sb[:], 0.0)
            junk_ps = _apsum("junk_ps")
            NJUNK = 7
            for jn in range(NJUNK):
                nc.tensor.matmul(
                    junk_ps[:], junk_sb[:, 0:128], junk_sb[:, 128:640],
                    start=(jn == 0), stop=(jn == NJUNK - 1),
                )

            # Phase A: per head, KT = (rQ)^T; per head-pair, V = r @ E^T
            # (two heads share the 512-wide moving operand). Kept in SBUF
            # as bf16. Emission order tracks DMA arrival (PE executes its
            # instruction stream in order): KT h0, KT h1, V pair0, ...
            def _kt(h, tc_order=(0, 1, 2, 3), split_tc0=False, pair_order=None):
                pairs = pair_order or [
                    (ipc, tcn) for ipc in range(2) for tcn in tc_order
                ]
                for ipc, tcn in pairs:
                        kt_ps = _apsum("kt_ps")
                        if split_tc0 and tcn == 0:
                            # 256-wide sub-groups so the first matmul only
                            # needs the first 64KB rt piece.
                            for half in range(2):
                                for ic in range(2):
                                    nc.tensor.matmul(
                                        kt_ps[:, half * 256 : (half + 1) * 256],
                                        q_sb[:, h, ic, ipc * 128 : (ipc + 1) * 128],
                                        rt_sb[:, ic, half * 256 : (half + 1) * 256],
                                        start=(half == 0 and ic == 0),
                                        stop=(half == 1 and ic == 1),
                                    )
                        else:
                            for ic in range(2):
                                nc.tensor.matmul(
                                    kt_ps[:],
                                    q_sb[:, h, ic, ipc * 128 : (ipc + 1) * 128],
                                    rt_sb[:, ic, tcn * ACH : (tcn + 1) * ACH],
                                    start=(ic == 0),
                                    stop=(ic == 1),
                                )
                        _cp(
                            kt_sb[:, h, ipc, tcn * ACH : (tcn + 1) * ACH],
                            kt_ps[:],
                        )

            def _v(hp):
                for ut in range(TQ):
                    v_ps = _apsum("v_ps")
                    for ic in range(2):
                        nc.tensor.matmul(
                            v_ps[:],
                            rt_sb[:, ic, ut * 128 : (ut + 1) * 128],
                            et_sb[:, hp, ic, :],
                            start=(ic == 0),
                            stop=(ic == 1),
                        )
                    _cp(v_sb[:, hp, ut, :], v_ps[:])

            # rt1 is the last rt chunk to land, so head 0 consumes it in
            # its final two psum groups; the second tc0 pass (data already
            # resident) runs while rt3/rt2 are still in flight.
            _kt(0, pair_order=[(0, 0), (1, 0), (0, 3), (0, 2), (1, 3), (1, 2),
                               (0, 1), (1, 1)])
            _kt(1, tc_order=(0, 3, 2, 1))
            _v(0)
            _kt(2)
            _kt(3)
            _v(1)

            # ---- Phase B: chunked linear attention -------------------
            # ST psum layout per head: cols 0:128 = u0 x t[0:128] (diag),
            # 128:256 = u0 x t[128:256] (full), 256:384 = u1 x t[128:256]
            # (diag).
            def _st(c, h):
                st_ps = psum.tile(
                    [128, 384], F32, tag="work", bufs=2, name="st_ps"
                )
                u0, u1 = 2 * c, 2 * c + 1
                for jt in range(2):
                    nc.tensor.matmul(
                        st_ps[:, 0:256],
                        rt_sb[:, jt, u0 * 128 : (u0 + 1) * 128],
                        kt_sb[:, h, jt, c * CB : c * CB + 256],
                        start=(jt == 0),
                        stop=False,
                    )
                for jt in range(2):
                    nc.tensor.matmul(
                        st_ps[:, 256:384],
                        rt_sb[:, jt, u1 * 128 : (u1 + 1) * 128],
                        kt_sb[:, h, jt, c * CB + 128 : c * CB + 256],
                        start=False,
                        stop=(jt == 1),
                    )
                st_sb = sbw.tile([128, 384], BF16, tag="st", bufs=8, name="st_sb")
                nc.vector.tensor_mul(
                    st_sb[:, 0:128], st_ps[:, 0:128], mask_sb[:]
                )
                nc.scalar.copy(st_sb[:, 128:256], st_ps[:, 128:256])
                nc.vector.tensor_mul(
                    st_sb[:, 256:384], st_ps[:, 256:384], mask_sb[:]
                )
                return st_sb

            def _inter(c, h, ot, first):
                hp, sh = h // 2, h % 2
                for jt in range(2):
                    for it in range(2):
                        nc.tensor.matmul(
                            ot[:, it * 256 : (it + 1) * 256],
                            m_sb[:, jt, hp, sh * 256 + it * 128 : sh * 256 + (it + 1) * 128],
                            kt_sb[:, h, jt, c * CB : (c + 1) * CB],
                            start=(first and jt == 0 and it == 0),
                            stop=False,
                        )

            def _ointra(c, h, ot, st_sb, first, last):
                hp, sh = h // 2, h % 2
                u0, u1 = 2 * c, 2 * c + 1
                for it in range(2):
                    nc.tensor.matmul(
                        ot[:, it * 256 : it * 256 + 256],
                        v_sb[:, hp, u0, sh * 256 + it * 128 : sh * 256 + (it + 1) * 128],
                        st_sb[:, 0:256],
                        start=(first and it == 0),
                        stop=False,
                    )
                    nc.tensor.matmul(
                        ot[:, it * 256 + 128 : it * 256 + 256],
                        v_sb[:, hp, u1, sh * 256 + it * 128 : sh * 256 + (it + 1) * 128],
                        st_sb[:, 256:384],
                        start=False,
                        stop=(last and it == 1),
                    )

            def _inter_last(c, h, otA, otB, first):
                # Last chunk: OT halves live in two separate PSUM banks so
                # the first half can drain to DRAM while the second half
                # is still accumulating (no PSUM bank-level WAR).
                hp, sh = h // 2, h % 2
                for jt in range(2):
                    for it in range(2):
                        tgt = otA if it == 0 else otB
                        nc.tensor.matmul(
                            tgt[:, 0:256],
                            m_sb[:, jt, hp, sh * 256 + it * 128 : sh * 256 + (it + 1) * 128],
                            kt_sb[:, h, jt, c * CB : (c + 1) * CB],
                            start=(first and jt == 0),
                            stop=False,
                        )

            for c in range(NCH):
                last = c == NCH - 1
                if not last:
                    ot = psum.tile([128, 512], F32, tag="ot", bufs=2, name="ot")
                else:
                    otA = psum.tile([128, 256], F32, tag="ot", bufs=2, name="otA")
                    otB = psum.tile([128, 256], F32, tag="ot", bufs=2, name="otB")
                st_tiles = [None] * 4
                # ST for h0/h1 first; h2/h3 interleave with inter so the
                # DVE/ACT mask ops have a window before intra needs them.
                st_tiles[0] = _st(c, 0)
                st_tiles[1] = _st(c, 1)
                if c > 0 and not last:
                    _inter(c, 0, ot, first=True)
                    st_tiles[2] = _st(c, 2)
                    _inter(c, 1, ot, first=False)
                    st_tiles[3] = _st(c, 3)
                    _inter(c, 2, ot, first=False)
                    _inter(c, 3, ot, first=False)
                elif last:
                    _inter_last(c, 0, otA, otB, first=True)
                    st_tiles[2] = _st(c, 2)
                    _inter_last(c, 1, otA, otB, first=False)
                    st_tiles[3] = _st(c, 3)
                    _inter_last(c, 2, otA, otB, first=False)
                    _inter_last(c, 3, otA, otB, first=False)
                else:
                    st_tiles[2] = _st(c, 2)
                    st_tiles[3] = _st(c, 3)

                if c < NCH - 1:
                    # G: accumulate this chunk into the persistent state,
                    # then snapshot state -> M (bf16 SBUF) for the next
                    # chunk's inter matmuls.
                    for hp in range(2):
                        for jt in range(2):
                            for us in range(2):
                                ut = 2 * c + us
                                nc.tensor.matmul(
                                    state[hp][jt][:],
                                    rn_sb[:, ut, jt * 128 : (jt + 1) * 128],
                                    v_sb[:, hp, ut, :],
                                    start=(c == 0 and us == 0),
                                    stop=(c == NCH - 2 and us == 1),
                                )
                    # hp0 copies on ACT (inter h0/h1 read them first),
                    # hp1 on DVE.
                    nc.scalar.copy(m_sb[:, 0, 0, :], state[0][0][:])
                    nc.scalar.copy(m_sb[:, 1, 0, :], state[0][1][:])
                    nc.vector.tensor_copy(m_sb[:, 0, 1, :], state[1][0][:])
                    nc.vector.tensor_copy(m_sb[:, 1, 1, :], state[1][1][:])

                if c < NCH - 1:
                    for h in range(4):
                        _ointra(
                            c, h, ot, st_tiles[h],
                            first=(c == 0 and h == 0),
                            last=(h == 3),
                        )
                    o_sb = outp.tile([128, 512], F32, tag="osb", name="o_sb")
                    nc.scalar.copy(o_sb[:], ot[:])
                    nc.sync.dma_start(
                        out=out_d[0, :, c * CB : (c + 1) * CB],
                        in_=o_sb[:, 0:256],
                    )
                    nc.gpsimd.dma_start(
                        out=out_d[1, :, c * CB : (c + 1) * CB],
                        in_=o_sb[:, 256:512],
                    )
                else:
                    # Last chunk: the end-of-kernel barrier waits on the
                    # final output DMA, so drain the first OT half (its
                    # own PSUM bank) as soon as its intra matmuls retire
                    # (it-major order) and spread the copies/DMAs over
                    # both PSUM-capable engines and four DMA queues.
                    o_sb = outp.tile([128, 512], F32, tag="osb", name="o_sb")
                    u0, u1 = 2 * c, 2 * c + 1
                    for it in range(2):
                        tgt = otA if it == 0 else otB
                        for h in range(4):
                            hp, sh = h // 2, h % 2
                            i0 = sh * 256 + it * 128
                            nc.tensor.matmul(
                                tgt[:, 0:256],
                                v_sb[:, hp, u0, i0 : i0 + 128],
                                st_tiles[h][:, 0:256],
                                start=False,
                                stop=False,
                            )
                            nc.tensor.matmul(
                                tgt[:, 128:256],
                                v_sb[:, hp, u1, i0 : i0 + 128],
                                st_tiles[h][:, 256:384],
                                start=False,
                                stop=(h == 3),
                            )
                    # The end-of-kernel drain waits on every DMA queue that
                    # was used; gpsimd's SWDGE drains slowest (~2.5us), so
                    # the final DMAs ride only the sync/scalar HWDGE queues.
                        if it == 0:
                            nc.vector.tensor_copy(
                                o_sb[:, 0:256], otA[:, 0:256]
                            )
                            nc.sync.dma_start(
                                out=out_d[0, :, c * CB : c * CB + 128],
                                in_=o_sb[:, 0:128],
                            )
                            nc.scalar.dma_start(
                                out=out_d[0, :, c * CB + 128 : (c + 1) * CB],
                                in_=o_sb[:, 128:256],
                            )
                    nc.vector.tensor_copy(o_sb[:, 256:512], otB[:, 0:256])
                    nc.scalar.dma_start(
                        out=out_d[1, :, c * CB : c * CB + 128],
                        in_=o_sb[:, 256:384],
                    )
                    nc.sync.dma_start(
                        out=out_d[1, :, c * CB + 128 : (c + 1) * CB],
                        in_=o_sb[:, 384:512],
                    )

    nc.compile()
    return nc


def _get_nc():
    if "nc" not in _CACHE:
        _CACHE["nc"] = _build_nc()
    return _CACHE["nc"]


def _make_in_maps(r_prime, Q, E):
    bf16 = ml_dtypes.bfloat16
    cmask = np.triu(np.ones((128, 128), np.float32))
    in_maps = []
    for core in range(N_CORES):
        b, hg = core // 2, core % 2
        r = r_prime[0, b]  # [2048, 256]
        # rt[p, ic, t] = r[t, ic*128+p]
        rt = np.ascontiguousarray(
            r.T.reshape(2, 128, N_T).transpose(1, 0, 2)
        ).astype(bf16)
        # rn[p, ut, j] = r[ut*128+p, j]
        rn = np.ascontiguousarray(
            r.reshape(TQ, 128, N_IN).transpose(1, 0, 2)
        ).astype(bf16)
        # q[p, h, ic, f] = Q[h, ic*128+p, f]
        qh = np.ascontiguousarray(
            Q[0, hg * 4 : hg * 4 + 4]
            .reshape(4, 2, 128, N_IN)
            .transpose(2, 0, 1, 3)
        ).astype(bf16)
        # et[p, hp, jc, sh*256+f] = E[2hp+sh].T[jc*128+p, f]
        eth = (
            E[0, hg * 4 : hg * 4 + 4]
            .transpose(0, 2, 1)  # [h, j, i']
            .reshape(2, 2, 2, 128, N_IN)  # [hp, sh, jc, p, f]
            .transpose(3, 0, 2, 1, 4)  # [p, hp, jc, sh, f]
            .reshape(128, 2, 2, ACH)
        )
        eth = np.ascontiguousarray(eth).astype(bf16)
        in_maps.append(
            {"rt": rt, "rn": rn, "q": qh, "et": eth, "cmask": cmask}
        )
    return in_maps


def _ensure_ntff_hook():
    """The container's `antenv` stub lacks `axon_hooks`, so the boot-time
    NTFF profile hook registration silently no-ops. Recreate it so
    trace=True yields exec_time_ns. Only used by the test harness."""
    import sys
    import types

    if "antenv.axon_hooks" not in sys.modules:
        import antenv

        mod = types.ModuleType("antenv.axon_hooks")
        state = {}
        mod.set_axon_ntff_profile_hook = lambda h: state.update(h=h)
        mod.get_axon_ntff_profile_hook = lambda: state.get("h")
        sys.modules["antenv.axon_hooks"] = mod
        antenv.axon_hooks = mod
    from antenv.axon_hooks import (
        get_axon_ntff_profile_hook,
        set_axon_ntff_profile_hook,
    )

    if get_axon_ntff_profile_hook() is None:
        from trn_agent_boot.trn_boot import _ntff_profile_via_ctypes

        set_axon_ntff_profile_hook(
            _ntff_profile_via_ctypes("/opt/axon/libaxon_pjrt.so")
        )


def _run(r_prime, Q, E, trace=False, trace_kwargs=None):
    from concourse.bass_utils import run_bass_kernel_spmd

    try:
        _ensure_ntff_hook()
    except Exception:
        pass  # profiling is optional; never block the actual run
    r_prime = np.asarray(r_prime, dtype=np.float32)
    Q = np.asarray(Q, dtype=np.float32)
    E = np.asarray(E, dtype=np.float32)
    in_maps = _make_in_maps(r_prime, Q, E)
    nc = _get_nc()
    kw = {}
    if trace:
        kw["trace"] = True
        if trace_kwargs:
            kw.update(trace_kwargs)
    res = run_bass_kernel_spmd(nc, in_maps, core_ids=list(range(N_CORES)), **kw)
    out = np.zeros((1, 4, N_T, N_IN), np.float32)
    for b in range(4):
        p0 = np.asarray(res.results[2 * b]["out"], np.float32).reshape(N_IN, N_T)
        p1 = np.asarray(res.results[2 * b + 1]["out"], np.float32).reshape(
            N_IN, N_T
        )
        out[0, b] = (p0 + p1).T
    return out, res


def kernel(r_prime, Q, E):
    out, _ = _run(r_prime, Q, E, trace=False)
    return out
